# Initial kernel scaffold
#
"""CRF-RNN mean-field kernel for Trainium2 (8 NeuronCores, data-parallel over T).

Math: reference computes, with x0 = inputs @ W_feat.T (T,N),
A[i,j] = sum_k kernels[i,j,k] W_lin[k], denom[i] = sum(W_feat) + 2*sum_j A[i,j],
the 4-step recurrence  x <- (x0 + 2 x A^T) / denom.
The recurrence is linear, so with D = diag(1/denom), B = 2 A^T D:
    x4 = x0 @ E,   E = D (I + B + B^2 + B^3) + B^4     (256x256, precomputed on-chip)
Each core: computes E redundantly from kernels, then streams its T-slice:
x0 via per-m MAC ops (DVE+Pool), PE transpose of x0, two accumulating fp32
matmuls against E, DMA out.
"""

import os
import sys

for _p in ("/opt/trn_rl_repo",):
    if _p not in sys.path and os.path.isdir(_p):
        sys.path.insert(0, _p)

import numpy as np

import concourse.bass as bass
import concourse.mybir as mybir
from concourse import bacc
from concourse.bass_utils import run_bass_kernel_spmd
from concourse.masks import make_identity
from concourse.tile import TileContext

F32 = mybir.dt.float32
AL = mybir.AluOpType
AX = mybir.AxisListType

T, N, M, K = 16384, 256, 8, 16
NCORES = 8
TL = T // NCORES  # 2048 rows per core
P = 128
NT = TL // P  # 16 t-tiles per core
NH = N // P  # 2 region halves


def _kernel_body(tc):
    nc = tc.nc
    inp = nc.dram_tensor("inp", (TL, N * M), F32, kind="ExternalInput").ap()
    kern = nc.dram_tensor("kern", (N, N * K), F32, kind="ExternalInput").ap()
    wf = nc.dram_tensor("wf", (1, M), F32, kind="ExternalInput").ap()
    wl = nc.dram_tensor("wl", (1, K), F32, kind="ExternalInput").ap()
    out = nc.dram_tensor("out", (TL, N), F32, kind="ExternalOutput").ap()

    from contextlib import ExitStack

    with ExitStack() as ctx:
        const = ctx.enter_context(tc.tile_pool(name="const", bufs=1))
        work = ctx.enter_context(tc.tile_pool(name="work", bufs=3))
        x0p = ctx.enter_context(tc.tile_pool(name="x0p", bufs=3))
        outp = ctx.enter_context(tc.tile_pool(name="outp", bufs=3))
        pst = ctx.enter_context(tc.tile_pool(name="pst", bufs=2, space="PSUM"))
        pso = ctx.enter_context(tc.tile_pool(name="pso", bufs=2, space="PSUM"))

        # ---------------- constants ----------------
        ident = const.tile([P, P], F32)
        make_identity(nc, ident[:])

        wf_row = const.tile([1, M], F32)
        nc.sync.dma_start(wf_row[:], wf[:, :])
        wf_sb = const.tile([P, M], F32)
        nc.gpsimd.partition_broadcast(wf_sb[:], wf_row[:])

        wl_row = const.tile([1, K], F32)
        nc.sync.dma_start(wl_row[:], wl[:, :])
        wl_sb = const.tile([P, K], F32)
        nc.gpsimd.partition_broadcast(wl_sb[:], wl_row[:])

        fw_sum = const.tile([P, 1], F32)
        nc.vector.tensor_reduce(fw_sum[:], wf_sb[:], axis=AX.X, op=AL.add)

        # ------------- precompute E (every core, identical) -------------
        # A[i,j] = sum_k kern[i, j, k] * wl[k]; two i-halves of 128 partitions.
        Bt = []  # Bt[h][i_loc, j] = B[j, h*128+i_loc] = 2*invd[i]*A[i,j]
        invd = []  # [128,1] per half, partition index = region index
        for h in range(NH):
            kt = const.tile([P, N * K], F32, tag=f"kern{h}", name=f"kern_sb{h}")
            nc.sync.dma_start(kt[:], kern[h * P : (h + 1) * P, :])
            kv = kt.rearrange("p (j k) -> p j k", k=K)
            Ah = const.tile([P, N], F32, tag=f"A{h}", name=f"A{h}")
            eng = nc.vector if h == 0 else nc.gpsimd
            eng.tensor_scalar_mul(Ah[:], kv[:, :, 0], wl_sb[:, 0:1])
            for k in range(1, K):
                eng.scalar_tensor_tensor(
                    Ah[:], kv[:, :, k], wl_sb[:, k : k + 1], Ah[:],
                    op0=AL.mult, op1=AL.add,
                )
            red = const.tile([P, 1], F32, tag=f"red{h}", name=f"red{h}")
            nc.vector.tensor_reduce(red[:], Ah[:], axis=AX.X, op=AL.add)
            den = const.tile([P, 1], F32, tag=f"den{h}", name=f"den{h}")
            nc.vector.scalar_tensor_tensor(
                den[:], red[:], 2.0, fw_sum[:], op0=AL.mult, op1=AL.add
            )
            inv = const.tile([P, 1], F32, tag=f"invd{h}", name=f"invd{h}")
            nc.vector.reciprocal(inv[:], den[:])
            invd.append(inv)
            inv2 = const.tile([P, 1], F32, tag=f"invd2{h}", name=f"invd2{h}")
            nc.vector.tensor_scalar_mul(inv2[:], inv[:], 2.0)
            Bth = const.tile([P, N], F32, tag=f"Bt{h}", name=f"Bt{h}")
            nc.scalar.mul(Bth[:], Ah[:], inv2[:, 0:1])
            Bt.append(Bth)

        # B1[jb][j_loc, i] = B[jb*128+j_loc, i]  (PE transpose of Bt blocks)
        B1 = [const.tile([P, N], F32, tag=f"B1{jb}", name=f"B1{jb}") for jb in range(NH)]
        for jb in range(NH):
            for ih in range(NH):
                pt = pst.tile([P, P], F32, tag="tr", name=f"trB{jb}{ih}")
                nc.tensor.transpose(pt[:], Bt[ih][:, jb * P : (jb + 1) * P], ident[:])
                nc.scalar.copy(B1[jb][:, ih * P : (ih + 1) * P], pt[:])

        # Powers: B_{n+1}[j,i] = sum_l Bt[l,j] * B_n[l,i]
        def mat_next(rhs_tiles, tag):
            res = [
                const.tile([P, N], F32, tag=f"{tag}{jb}", name=f"{tag}{jb}")
                for jb in range(NH)
            ]
            for jb in range(NH):
                ps = pso.tile([P, N], F32, tag="pw", name=f"pw{tag}{jb}")
                for lh in range(NH):
                    nc.tensor.matmul(
                        ps[:],
                        Bt[lh][:, jb * P : (jb + 1) * P],
                        rhs_tiles[lh][:],
                        start=(lh == 0),
                        stop=(lh == NH - 1),
                    )
                nc.scalar.copy(res[jb][:], ps[:])
            return res

        B2 = mat_next(B1, "B2")
        B3 = mat_next(B2, "B3")
        B4 = mat_next(B3, "B4")

        # E[jb] = invd ⊙ (I + B1 + B2 + B3)[jb] + B4[jb]
        E = [const.tile([P, N], F32, tag=f"E{jb}", name=f"E{jb}") for jb in range(NH)]
        for jb in range(NH):
            s = E[jb]
            nc.vector.tensor_add(s[:], B1[jb][:], B2[jb][:])
            nc.vector.tensor_add(s[:], s[:], B3[jb][:])
            nc.vector.tensor_add(
                s[:, jb * P : (jb + 1) * P], s[:, jb * P : (jb + 1) * P], ident[:]
            )
            nc.scalar.mul(s[:], s[:], invd[jb][:, 0:1])
            nc.vector.tensor_add(s[:], s[:], B4[jb][:])

        # ---------------- main loop over t-tiles ----------------
        NSPLIT = 5  # m-indices 0..4 on DVE, 5..7 on GpSimd
        for tt in range(NT):
            it = work.tile([P, N * M], F32, tag="in", name=f"in{tt}")
            nc.sync.dma_start(it[:], inp[tt * P : (tt + 1) * P, :])
            iv = it.rearrange("p (j m) -> p j m", m=M)

            x0a = x0p.tile([P, N], F32, tag="x0a", name=f"x0a{tt}")
            nc.vector.tensor_scalar_mul(x0a[:], iv[:, :, 0], wf_sb[:, 0:1])
            for m in range(1, NSPLIT):
                nc.vector.scalar_tensor_tensor(
                    x0a[:], iv[:, :, m], wf_sb[:, m : m + 1], x0a[:],
                    op0=AL.mult, op1=AL.add,
                )
            x0b = x0p.tile([P, N], F32, tag="x0b", name=f"x0b{tt}")
            nc.gpsimd.tensor_scalar_mul(x0b[:], iv[:, :, NSPLIT], wf_sb[:, NSPLIT : NSPLIT + 1])
            for m in range(NSPLIT + 1, M):
                nc.gpsimd.scalar_tensor_tensor(
                    x0b[:], iv[:, :, m], wf_sb[:, m : m + 1], x0b[:],
                    op0=AL.mult, op1=AL.add,
                )
            x0 = x0p.tile([P, N], F32, tag="x0", name=f"x0{tt}")
            nc.vector.tensor_add(x0[:], x0a[:], x0b[:])

            x0T = []
            for jb in range(NH):
                pt = pst.tile([P, P], F32, tag="tr", name=f"tr{tt}_{jb}")
                nc.tensor.transpose(pt[:], x0[:, jb * P : (jb + 1) * P], ident[:])
                xs = x0p.tile([P, P], F32, tag=f"x0T{jb}", name=f"x0T{tt}_{jb}")
                nc.scalar.copy(xs[:], pt[:])
                x0T.append(xs)

            po = pso.tile([P, N], F32, tag="out", name=f"po{tt}")
            for jb in range(NH):
                nc.tensor.matmul(
                    po[:], x0T[jb][:], E[jb][:], start=(jb == 0), stop=(jb == NH - 1)
                )
            ot = outp.tile([P, N], F32, tag="ot", name=f"ot{tt}")
            nc.scalar.copy(ot[:], po[:])
            nc.sync.dma_start(out[tt * P : (tt + 1) * P, :], ot[:])


_NC_CACHE = None


def _build():
    global _NC_CACHE
    if _NC_CACHE is not None:
        return _NC_CACHE
    nc = bacc.Bacc(
        "TRN2",
        target_bir_lowering=False,
        debug=False,
        enable_asserts=False,
        num_devices=NCORES,
    )
    with TileContext(nc) as tc:
        _kernel_body(tc)
    nc.compile()
    _NC_CACHE = nc
    return nc


def kernel(inputs, kernels, W_feat, W_lin, trace=False):
    inp = np.ascontiguousarray(np.asarray(inputs, dtype=np.float32).reshape(T, N * M))
    kr = np.ascontiguousarray(np.asarray(kernels, dtype=np.float32).reshape(N, N * K))
    wf = np.ascontiguousarray(np.asarray(W_feat, dtype=np.float32).reshape(1, M))
    wl = np.ascontiguousarray(np.asarray(W_lin, dtype=np.float32).reshape(1, K))

    nc = _build()
    in_maps = [
        {"inp": inp[c * TL : (c + 1) * TL], "kern": kr, "wf": wf, "wl": wl}
        for c in range(NCORES)
    ]
    res = run_bass_kernel_spmd(nc, in_maps, core_ids=list(range(NCORES)), trace=trace)
    outs = [res.results[c]["out"] for c in range(NCORES)]
    full = np.concatenate(outs, axis=0).reshape(T, N, 1)
    if trace:
        kernel.last_exec_time_ns = res.exec_time_ns
        kernel.last_results = res
    return full


# revision 8
# speedup vs baseline: 5.0403x; 5.0403x over previous
"""CRF-RNN mean-field kernel for Trainium2 (8 NeuronCores, data-parallel over T).

Math: reference computes, with x0 = inputs @ W_feat.T (T,N),
A[i,j] = sum_k kernels[i,j,k] W_lin[k], denom[i] = sum(W_feat) + 2*sum_j A[i,j],
the 4-step recurrence  x <- (x0 + 2 x A^T) / denom.
The recurrence is linear, so with D = diag(1/denom), B = 2 A^T D:
    x4 = x0 @ E,   E = D (I + B + B^2 + B^3) + B^4     (256x256, precomputed on-chip)
Each core: computes E redundantly from kernels, then streams its T-slice:
x0 via per-m MAC ops (DVE+Pool), PE transpose of x0, two accumulating fp32
matmuls against E, DMA out.
"""

import os
import sys

for _p in ("/opt/trn_rl_repo",):
    if _p not in sys.path and os.path.isdir(_p):
        sys.path.insert(0, _p)

import numpy as np

import concourse.bass as bass
import concourse.mybir as mybir
from concourse import bacc
from concourse.bass_utils import run_bass_kernel_spmd
from concourse.masks import make_identity
from concourse.tile import TileContext

F32 = mybir.dt.float32
AL = mybir.AluOpType
AX = mybir.AxisListType

T, N, M, K = 16384, 256, 8, 16
NCORES = 8
TL = T // NCORES  # 2048 rows per core
P = 128
NT = TL // P  # 16 t-tiles per core
NH = N // P  # 2 region halves


def _kernel_body(tc, inp, kern, wf, wl, out):
    nc = tc.nc

    from contextlib import ExitStack

    with ExitStack() as ctx:
        const = ctx.enter_context(tc.tile_pool(name="const", bufs=1))
        work = ctx.enter_context(tc.tile_pool(name="work", bufs=3))
        x0p = ctx.enter_context(tc.tile_pool(name="x0p", bufs=3))
        outp = ctx.enter_context(tc.tile_pool(name="outp", bufs=3))
        pst = ctx.enter_context(tc.tile_pool(name="pst", bufs=2, space="PSUM"))
        pso = ctx.enter_context(tc.tile_pool(name="pso", bufs=2, space="PSUM"))

        # ---------------- constants ----------------
        ident = const.tile([P, P], F32)
        make_identity(nc, ident[:])

        wf_row = const.tile([1, M], F32)
        nc.sync.dma_start(wf_row[:], wf[:, :])
        wf_sb = const.tile([P, M], F32)
        nc.gpsimd.partition_broadcast(wf_sb[:], wf_row[:])

        wl_row = const.tile([1, K], F32)
        nc.sync.dma_start(wl_row[:], wl[:, :])
        wl_sb = const.tile([P, K], F32)
        nc.gpsimd.partition_broadcast(wl_sb[:], wl_row[:])

        fw_sum = const.tile([P, 1], F32)
        nc.vector.tensor_reduce(fw_sum[:], wf_sb[:], axis=AX.X, op=AL.add)

        # ------------- precompute E (every core, identical) -------------
        # A[i,j] = sum_k kern[i, j, k] * wl[k]; two i-halves of 128 partitions.
        Bt = []  # Bt[h][i_loc, j] = B[j, h*128+i_loc] = 2*invd[i]*A[i,j]
        invd = []  # [128,1] per half, partition index = region index
        for h in range(NH):
            kt = const.tile([P, N * K], F32, tag=f"kern{h}", name=f"kern_sb{h}")
            nc.sync.dma_start(kt[:], kern[h * P : (h + 1) * P, :])
            kv = kt.rearrange("p (j k) -> p j k", k=K)
            Ah = const.tile([P, N], F32, tag=f"A{h}", name=f"A{h}")
            nc.vector.tensor_scalar_mul(Ah[:], kv[:, :, 0], wl_sb[:, 0:1])
            for k in range(1, K):
                nc.vector.scalar_tensor_tensor(
                    Ah[:], kv[:, :, k], wl_sb[:, k : k + 1], Ah[:],
                    op0=AL.mult, op1=AL.add,
                )
            red = const.tile([P, 1], F32, tag=f"red{h}", name=f"red{h}")
            nc.vector.tensor_reduce(red[:], Ah[:], axis=AX.X, op=AL.add)
            den = const.tile([P, 1], F32, tag=f"den{h}", name=f"den{h}")
            nc.vector.scalar_tensor_tensor(
                den[:], red[:], 2.0, fw_sum[:], op0=AL.mult, op1=AL.add
            )
            inv = const.tile([P, 1], F32, tag=f"invd{h}", name=f"invd{h}")
            nc.vector.reciprocal(inv[:], den[:])
            invd.append(inv)
            inv2 = const.tile([P, 1], F32, tag=f"invd2{h}", name=f"invd2{h}")
            nc.vector.tensor_scalar_mul(inv2[:], inv[:], 2.0)
            Bth = const.tile([P, N], F32, tag=f"Bt{h}", name=f"Bt{h}")
            nc.scalar.mul(Bth[:], Ah[:], inv2[:, 0:1])
            Bt.append(Bth)

        # B1[jb][j_loc, i] = B[jb*128+j_loc, i]  (PE transpose of Bt blocks)
        B1 = [const.tile([P, N], F32, tag=f"B1{jb}", name=f"B1{jb}") for jb in range(NH)]
        for jb in range(NH):
            for ih in range(NH):
                pt = pst.tile([P, P], F32, tag="tr", name=f"trB{jb}{ih}")
                nc.tensor.transpose(pt[:], Bt[ih][:, jb * P : (jb + 1) * P], ident[:])
                nc.scalar.copy(B1[jb][:, ih * P : (ih + 1) * P], pt[:])

        # Powers: B_{n+1}[j,i] = sum_l Bt[l,j] * B_n[l,i]
        def mat_next(rhs_tiles, tag):
            res = [
                const.tile([P, N], F32, tag=f"{tag}{jb}", name=f"{tag}{jb}")
                for jb in range(NH)
            ]
            for jb in range(NH):
                ps = pso.tile([P, N], F32, tag="pw", name=f"pw{tag}{jb}")
                for lh in range(NH):
                    nc.tensor.matmul(
                        ps[:],
                        Bt[lh][:, jb * P : (jb + 1) * P],
                        rhs_tiles[lh][:],
                        start=(lh == 0),
                        stop=(lh == NH - 1),
                    )
                nc.scalar.copy(res[jb][:], ps[:])
            return res

        B2 = mat_next(B1, "B2")
        B3 = mat_next(B2, "B3")
        B4 = mat_next(B3, "B4")

        # E[jb] = invd ⊙ (I + B1 + B2 + B3)[jb] + B4[jb]
        E = [const.tile([P, N], F32, tag=f"E{jb}", name=f"E{jb}") for jb in range(NH)]
        for jb in range(NH):
            s = E[jb]
            nc.vector.tensor_add(s[:], B1[jb][:], B2[jb][:])
            nc.vector.tensor_add(s[:], s[:], B3[jb][:])
            nc.vector.tensor_add(
                s[:, jb * P : (jb + 1) * P], s[:, jb * P : (jb + 1) * P], ident[:]
            )
            nc.scalar.mul(s[:], s[:], invd[jb][:, 0:1])
            nc.vector.tensor_add(s[:], s[:], B4[jb][:])

        # ---------------- main loop over t-tiles ----------------
        for tt in range(NT):
            it = work.tile([P, N * M], F32, tag="in", name=f"in{tt}")
            nc.sync.dma_start(it[:], inp[tt * P : (tt + 1) * P, :])
            iv = it.rearrange("p (j m) -> p j m", m=M)

            x0 = x0p.tile([P, N], F32, tag="x0", name=f"x0{tt}")
            nc.vector.tensor_scalar_mul(x0[:], iv[:, :, 0], wf_sb[:, 0:1])
            for m in range(1, M):
                nc.vector.scalar_tensor_tensor(
                    x0[:], iv[:, :, m], wf_sb[:, m : m + 1], x0[:],
                    op0=AL.mult, op1=AL.add,
                )

            x0T = []
            for jb in range(NH):
                pt = pst.tile([P, P], F32, tag="tr", name=f"tr{tt}_{jb}")
                nc.tensor.transpose(pt[:], x0[:, jb * P : (jb + 1) * P], ident[:])
                xs = x0p.tile([P, P], F32, tag=f"x0T{jb}", name=f"x0T{tt}_{jb}")
                nc.scalar.copy(xs[:], pt[:])
                x0T.append(xs)

            po = pso.tile([P, N], F32, tag="out", name=f"po{tt}")
            for jb in range(NH):
                nc.tensor.matmul(
                    po[:], x0T[jb][:], E[jb][:], start=(jb == 0), stop=(jb == NH - 1)
                )
            ot = outp.tile([P, N], F32, tag="ot", name=f"ot{tt}")
            nc.scalar.copy(ot[:], po[:])
            nc.sync.dma_start(out[tt * P : (tt + 1) * P, :], ot[:])


_NC_CACHE = {}


def _build(bodies=1):
    if bodies in _NC_CACHE:
        return _NC_CACHE[bodies]
    nc = bacc.Bacc(
        "TRN2",
        target_bir_lowering=False,
        debug=False,
        enable_asserts=False,
        num_devices=NCORES,
    )
    inp = nc.dram_tensor("inp", (TL, N * M), F32, kind="ExternalInput").ap()
    kern = nc.dram_tensor("kern", (N, N * K), F32, kind="ExternalInput").ap()
    wf = nc.dram_tensor("wf", (1, M), F32, kind="ExternalInput").ap()
    wl = nc.dram_tensor("wl", (1, K), F32, kind="ExternalInput").ap()
    out = nc.dram_tensor("out", (TL, N), F32, kind="ExternalOutput").ap()
    with TileContext(nc) as tc:
        for _ in range(bodies):
            _kernel_body(tc, inp, kern, wf, wl, out)
    nc.compile()
    _NC_CACHE[bodies] = nc
    return nc


def kernel(inputs, kernels, W_feat, W_lin, trace=False):
    inp = np.ascontiguousarray(np.asarray(inputs, dtype=np.float32).reshape(T, N * M))
    kr = np.ascontiguousarray(np.asarray(kernels, dtype=np.float32).reshape(N, N * K))
    wf = np.ascontiguousarray(np.asarray(W_feat, dtype=np.float32).reshape(1, M))
    wl = np.ascontiguousarray(np.asarray(W_lin, dtype=np.float32).reshape(1, K))

    nc = _build(1)
    in_maps = [
        {"inp": inp[c * TL : (c + 1) * TL], "kern": kr, "wf": wf, "wl": wl}
        for c in range(NCORES)
    ]
    res = run_bass_kernel_spmd(nc, in_maps, core_ids=list(range(NCORES)), trace=trace)
    outs = [res.results[c]["out"] for c in range(NCORES)]
    full = np.concatenate(outs, axis=0).reshape(T, N, 1)
    if trace:
        kernel.last_exec_time_ns = res.exec_time_ns
        kernel.last_results = res
    return full


def _pjrt_callable(nc):
    """Build a jit(shard_map(bass_exec)) callable + device-resident input list,
    mirroring bass2jax.run_bass_via_pjrt (no donation: outputs reallocated)."""
    import jax
    from jax.sharding import Mesh, NamedSharding, PartitionSpec
    from jax.experimental.shard_map import shard_map

    from concourse.bass2jax import (
        _bass_exec_p,
        install_neuronx_cc_hook,
        partition_id_tensor,
    )

    install_neuronx_cc_hook()
    partition_name = nc.partition_id_tensor.name if nc.partition_id_tensor else None
    in_names, out_names, out_avals = [], [], []
    for alloc in nc.m.functions[0].allocations:
        if not isinstance(alloc, mybir.MemoryLocationSet):
            continue
        name = alloc.memorylocations[0].name
        if alloc.kind == "ExternalInput":
            if name != partition_name:
                in_names.append(name)
        elif alloc.kind == "ExternalOutput":
            out_names.append(name)
            out_avals.append(
                jax.core.ShapedArray(tuple(alloc.tensor_shape), mybir.dt.np(alloc.dtype))
            )
    all_in = list(in_names) + list(out_names)
    if partition_name is not None:
        all_in.append(partition_name)
    all_in = tuple(all_in)

    def _body(*args):
        operands = list(args)
        if partition_name is not None:
            operands.append(partition_id_tensor())
        return tuple(
            _bass_exec_p.bind(
                *operands,
                out_avals=tuple(out_avals),
                in_names=all_in,
                out_names=tuple(out_names),
                lowering_input_output_aliases=(),
                sim_require_finite=True,
                sim_require_nnan=True,
                nc=nc,
            )
        )

    devices = jax.devices()[:NCORES]
    mesh = Mesh(np.asarray(devices), ("core",))
    nin = len(in_names) + len(out_names)
    fn = jax.jit(
        shard_map(
            _body,
            mesh=mesh,
            in_specs=(PartitionSpec("core"),) * nin,
            out_specs=(PartitionSpec("core"),) * len(out_names),
            check_rep=False,
        )
    )
    sh = NamedSharding(mesh, PartitionSpec("core"))
    return fn, in_names, out_names, out_avals, sh


def bench(bodies_list=(1, 4), reps=30):
    """Time the NEFF via repeated dispatch of R-body program variants.
    Marginal per-body time = (t(R2)-t(R1))/(R2-R1) cancels dispatch overhead."""
    import time

    import jax

    rng = np.random.default_rng(0)
    inp = rng.standard_normal((T, N * M), dtype=np.float32)
    kr = rng.random((N, N * K), dtype=np.float32)
    wf = (rng.random((1, M), dtype=np.float32) * 0.01).astype(np.float32)
    wl = (rng.random((1, K), dtype=np.float32) * 0.01).astype(np.float32)
    vals = {"inp": inp, "kern": kr, "wf": wf, "wl": wl}

    times = {}
    for bodies in bodies_list:
        nc = _build(bodies)
        fn, in_names, out_names, out_avals, sh = _pjrt_callable(nc)
        cat = {
            "inp": inp,
            "kern": np.concatenate([kr] * NCORES, 0),
            "wf": np.concatenate([wf] * NCORES, 0),
            "wl": np.concatenate([wl] * NCORES, 0),
        }
        args = [jax.device_put(cat[n], sh) for n in in_names]
        args += [
            jax.device_put(np.zeros((NCORES * a.shape[0], *a.shape[1:]), a.dtype), sh)
            for a in out_avals
        ]
        o = fn(*args)
        jax.block_until_ready(o)  # warm (NEFF compile happens here)
        ts = []
        for _ in range(reps):
            t0 = time.perf_counter()
            o = fn(*args)
            jax.block_until_ready(o)
            ts.append(time.perf_counter() - t0)
        ts.sort()
        med = ts[len(ts) // 2]
        times[bodies] = med
        print(f"bodies={bodies}: median dispatch {med*1e6:.1f} us  (min {ts[0]*1e6:.1f})")
    bs = sorted(times)
    if len(bs) >= 2:
        r1, r2 = bs[0], bs[-1]
        marginal = (times[r2] - times[r1]) / (r2 - r1)
        print(f"marginal per-body time: {marginal*1e9:.0f} ns")
        return marginal * 1e9, times
    return None, times
